# revision 8
# baseline (speedup 1.0000x reference)
"""Trainium2 Bass kernel for AbsolutePool (text-video attention pooling).

reference:
    scores[a,b,v] = sum_d text[a,d] * video[b,v,d]
    w = softmax(scores / 5.0, axis=v)
    out[a,b,d]   = sum_v w[a,b,v] * video[b,v,d]

Shapes: text [A=512, D=512] f32, video [B=512, V=32, D=512] f32,
out [512, 512, 512] f32 (512 MB -> memory-bound on the output write).

Sharding: B is split across the 8 cores (64 b's each).  Each core reads its
4 MB video shard (twice: natural + host-transposed layouts), the replicated
1 MB transposed text, and writes a 64 MB output shard out[:, b_lo:b_hi, :].
The full output is re-assembled on host with one concatenate along axis 1.

Per-core plan (all matmuls in float32r = full-rate FP22 PE mode):
  groups of 4 b's (4*32 v = 128 partitions):
    mm1   scores[(b,v)=128, a=512]  = video_t.T @ text_t   (4 accum steps over d)
    ACT   e = exp(scores / 5)                       PSUM -> SBUF
    mmZ   Z[a=128chunk, 4b] = e.T @ blockdiag_ones  (sum over v per b)
    DVE   R = 1/Z
    mm2   pooled[a=128chunk, d=512] = e_b.T @ video_b   (K = 32 v's)
    DVE/ACT  out_sbuf = pooled * R[a]   (normalize fused into PSUM eviction)
    DMA   out[a_chunk, 4 b's, :]  (1 MB contiguous-per-row stores)
"""

import numpy as np

import concourse.bass as bass
import concourse.bacc as bacc
import concourse.tile as tile
from concourse import mybir
from concourse.bass_utils import run_bass_kernel_spmd

N_CORES = 8
A = 512
D = 512
B = 512
V = 32
B_SH = B // N_CORES            # 64 b's per core
NG = B_SH // 4                 # 16 groups of 4 b's
TEMP = 5.0

F32 = mybir.dt.float32
F32R = mybir.dt.float32r

# which of the 16 (ac, bp) evictions per group go to the scalar engine
# (the rest go to the vector engine; ACT also runs the exp)
ACT_PAIRS = frozenset({1, 4, 6})  # 3 of 8 eviction pairs per group go to ACT


def build_nc(n_groups: int = NG) -> bass.Bass:
    bsh = 4 * n_groups
    nc = bacc.Bacc(None, target_bir_lowering=False, debug=False)

    text_t = nc.dram_tensor("text_t", [D, A], F32, kind="ExternalInput")
    video_nat = nc.dram_tensor("video_nat", [bsh * V, D], F32, kind="ExternalInput")
    video_t = nc.dram_tensor("video_t", [D, bsh * V], F32, kind="ExternalInput")
    out = nc.dram_tensor("out", [A, bsh, D], F32, kind="ExternalOutput")

    ones_np = np.zeros((128, 4), dtype=np.float32)
    for j in range(4):
        ones_np[32 * j : 32 * (j + 1), j] = 1.0
    ones_dram = nc.inline_tensor(ones_np, name="ones_bd")

    with tile.TileContext(nc) as tc:
        with (
            tc.tile_pool(name="const", bufs=1) as cpool,
            tc.tile_pool(name="exp", bufs=3) as epool,
            tc.tile_pool(name="rpool", bufs=3) as rpool,
            tc.tile_pool(name="outp", bufs=8) as opool,
            tc.tile_pool(name="ps_s", bufs=2, space="PSUM") as ps_s,
            tc.tile_pool(name="ps_z", bufs=1, space="PSUM") as ps_z,
            tc.tile_pool(name="ps_p", bufs=2, space="PSUM") as ps_p,
        ):
            # resident inputs ---------------------------------------------
            tt = cpool.tile([128, 4, A], F32R, tag="tt")        # text_t  1 MB
            nc.sync.dma_start(
                tt[:], text_t[:].rearrange("(c p) a -> p c a", p=128).bitcast(F32R)
            )
            vt = cpool.tile([128, 4, bsh * V], F32R, tag="vt")  # video_t 4 MB
            vt_src = video_t[:].rearrange("(c p) n -> p c n", p=128).bitcast(F32R)
            gq = max(1, n_groups // 4)
            for q in range(0, n_groups, gq):
                for dc in range(4):
                    nc.sync.dma_start(
                        vt[:, dc, 128 * q : 128 * (q + gq)],
                        vt_src[:, dc, 128 * q : 128 * (q + gq)],
                    )
            vn = cpool.tile([128, n_groups, D], F32R, tag="vn") # video   4 MB
            vn_src = video_nat[:].rearrange("(g p) d -> p g d", p=128).bitcast(F32R)
            for gi in range(n_groups):
                nc.sync.dma_start(vn[:, gi, :], vn_src[:, gi, :])
            ob = cpool.tile([128, 4], F32R, tag="ones")
            nc.sync.dma_start(ob[:], ones_dram[:].bitcast(F32R))

            def stage1(g):
                # mm1: scores[(b,v)=128, a=512], accumulate over 4 d-chunks
                scores = ps_s.tile([128, A], F32, tag="scores")
                for dc in range(4):
                    nc.tensor.matmul(
                        scores[:],
                        lhsT=vt[:, dc, 128 * g : 128 * (g + 1)],
                        rhs=tt[:, dc, :],
                        start=(dc == 0),
                        stop=(dc == 3),
                    )
                # e = exp(scores / TEMP)
                expt = epool.tile([128, A], F32R, tag="expt")
                nc.scalar.activation(
                    expt[:],
                    scores[:],
                    mybir.ActivationFunctionType.Exp,
                    bias=0.0,
                    scale=1.0 / TEMP,
                )
                return expt

            def stage2(g, expt):
                # Z[a, b'] = sum_v e[(b',v), a] via block-diagonal ones
                z = ps_z.tile([128, 16], F32, tag="z")
                for ac in range(4):
                    nc.tensor.matmul(
                        z[:, 4 * ac : 4 * (ac + 1)],
                        lhsT=expt[:, 128 * ac : 128 * (ac + 1)],
                        rhs=ob[:],
                        start=True,
                        stop=True,
                    )
                r = rpool.tile([128, 16], F32, tag="r")
                nc.vector.reciprocal(r[:], z[:])

                # mm2 (pairs of b's share a 2-bank PSUM tile) + fused
                # normalize/evict + 0.5 MB stores
                for ac in range(4):
                    for h in range(2):
                        pooled = ps_p.tile([128, 2, D], F32, tag="pooled")
                        for k in range(2):
                            bp = 2 * h + k
                            nc.tensor.matmul(
                                pooled[:, k, :],
                                lhsT=expt[
                                    32 * bp : 32 * (bp + 1),
                                    128 * ac : 128 * (ac + 1),
                                ],
                                rhs=vn[32 * bp : 32 * (bp + 1), g, :],
                                start=True,
                                stop=True,
                                tile_position=(32 * bp, 0),
                            )
                        ot = opool.tile([128, 2, D], F32, tag="ot")
                        j = (g * 8 + ac * 2 + h) % 8
                        if j in ACT_PAIRS:
                            # scalar engine: two per-partition-scaled copies
                            for k in range(2):
                                i = 4 * ac + 2 * h + k
                                nc.scalar.mul(
                                    ot[:, k, :], pooled[:, k, :], mul=r[:, i : i + 1]
                                )
                        else:
                            # vector engine: one op over both banks with the
                            # two 1/Z columns broadcast along d
                            i = 4 * ac + 2 * h
                            nc.vector.tensor_mul(
                                ot[:],
                                pooled[:],
                                r[:, i : i + 2].to_broadcast((128, 2, D)),
                            )
                        nc.sync.dma_start(
                            out[
                                128 * ac : 128 * (ac + 1),
                                4 * g + 2 * h : 4 * g + 2 * h + 2,
                                :,
                            ],
                            ot[:],
                        )

            prev = None
            for g in range(n_groups + 1):
                cur = stage1(g) if g < n_groups else None
                if prev is not None:
                    stage2(g - 1, prev)
                prev = cur
    nc.compile()
    return nc


_NC_CACHE: dict[int, bass.Bass] = {}


def _get_nc(n_groups: int = NG) -> bass.Bass:
    if n_groups not in _NC_CACHE:
        _NC_CACHE[n_groups] = build_nc(n_groups)
    return _NC_CACHE[n_groups]


def make_in_maps(text_features: np.ndarray, video_features: np.ndarray):
    text_t = np.ascontiguousarray(np.asarray(text_features, dtype=np.float32).T)
    video_features = np.asarray(video_features, dtype=np.float32)
    in_maps = []
    for c in range(N_CORES):
        vsh = video_features[B_SH * c : B_SH * (c + 1)].reshape(B_SH * V, D)
        in_maps.append(
            {
                "text_t": text_t,
                "video_nat": np.ascontiguousarray(vsh),
                "video_t": np.ascontiguousarray(vsh.T),
            }
        )
    return in_maps


def kernel(text_features: np.ndarray, video_features: np.ndarray) -> np.ndarray:
    nc = _get_nc()
    in_maps = make_in_maps(text_features, video_features)
    res = run_bass_kernel_spmd(nc, in_maps, list(range(N_CORES)))
    return np.concatenate([res.results[c]["out"] for c in range(N_CORES)], axis=1)


# revision 10
# speedup vs baseline: 1.2714x; 1.2714x over previous
"""Trainium2 Bass kernel for AbsolutePool (text-video attention pooling).

reference:
    scores[a,b,v] = sum_d text[a,d] * video[b,v,d]
    w = softmax(scores / 5.0, axis=v)
    out[a,b,d]   = sum_v w[a,b,v] * video[b,v,d]

Shapes: text [A=512, D=512] f32, video [B=512, V=32, D=512] f32,
out [512, 512, 512] f32 (512 MB -> memory-bound on the output write).

Sharding: B is split across the 8 cores (64 b's each).  Each core reads its
4 MB video shard (twice: natural + host-transposed layouts), the replicated
1 MB transposed text, and writes a 64 MB output shard out[:, b_lo:b_hi, :].
The full output is re-assembled on host with one concatenate along axis 1.

Per-core plan (all matmuls in float32r = full-rate FP22 PE mode):
  groups of 4 b's (4*32 v = 128 partitions):
    mm1   scores[(b,v)=128, a=512]  = video_t.T @ text_t   (4 accum steps over d)
    ACT   e = exp(scores / 5)                       PSUM -> SBUF
    mmZ   Z[a=128chunk, 4b] = e.T @ blockdiag_ones  (sum over v per b)
    DVE   R = 1/Z
    mm2   pooled[a=128chunk, d=512] = e_b.T @ video_b   (K = 32 v's)
    DVE/ACT  out_sbuf = pooled * R[a]   (normalize fused into PSUM eviction)
    DMA   out[a_chunk, 4 b's, :]  (1 MB contiguous-per-row stores)
"""

import numpy as np

import concourse.bass as bass
import concourse.bacc as bacc
import concourse.tile as tile
from concourse import mybir
from concourse.bass_utils import run_bass_kernel_spmd

N_CORES = 8
A = 512
D = 512
B = 512
V = 32
B_SH = B // N_CORES            # 64 b's per core
NG = B_SH // 4                 # 16 groups of 4 b's
TEMP = 5.0

F32 = mybir.dt.float32
F32R = mybir.dt.float32r

# which of the 16 (ac, bp) evictions per group go to the scalar engine
# (the rest go to the vector engine; ACT also runs the exp)
ACT_PAIRS = frozenset({1, 4, 6})  # 3 of 8 eviction pairs per group go to ACT


def build_nc(n_groups: int = NG) -> bass.Bass:
    bsh = 4 * n_groups
    nc = bacc.Bacc(None, target_bir_lowering=False, debug=False)

    text_t = nc.dram_tensor("text_t", [D, A], F32, kind="ExternalInput")
    video_nat = nc.dram_tensor("video_nat", [bsh * V, D], F32, kind="ExternalInput")
    video_t = nc.dram_tensor("video_t", [D, bsh * V], F32, kind="ExternalInput")
    out = nc.dram_tensor("out", [A, bsh, D], F32, kind="ExternalOutput")

    ones_np = np.zeros((128, 4), dtype=np.float32)
    for j in range(4):
        ones_np[32 * j : 32 * (j + 1), j] = 1.0
    ones_dram = nc.inline_tensor(ones_np, name="ones_bd")

    with tile.TileContext(nc) as tc:
        with (
            tc.tile_pool(name="const", bufs=1) as cpool,
            tc.tile_pool(name="exp", bufs=3) as epool,
            tc.tile_pool(name="rpool", bufs=3) as rpool,
            tc.tile_pool(name="outp", bufs=8) as opool,
            tc.tile_pool(name="ps_s", bufs=3, space="PSUM") as ps_s,
            tc.tile_pool(name="ps_z", bufs=1, space="PSUM") as ps_z,
            tc.tile_pool(name="ps_p", bufs=4, space="PSUM") as ps_p,
        ):
            # resident inputs ---------------------------------------------
            tt = cpool.tile([128, 4, A], F32R, tag="tt")        # text_t  1 MB
            nc.sync.dma_start(
                tt[:], text_t[:].rearrange("(c p) a -> p c a", p=128).bitcast(F32R)
            )
            vt = cpool.tile([128, 4, bsh * V], F32R, tag="vt")  # video_t 4 MB
            vt_src = video_t[:].rearrange("(c p) n -> p c n", p=128).bitcast(F32R)
            gq = max(1, n_groups // 4)
            for q in range(0, n_groups, gq):
                for dc in range(4):
                    nc.sync.dma_start(
                        vt[:, dc, 128 * q : 128 * (q + gq)],
                        vt_src[:, dc, 128 * q : 128 * (q + gq)],
                    )
            vn = cpool.tile([128, n_groups, D], F32R, tag="vn") # video   4 MB
            vn_src = video_nat[:].rearrange("(g p) d -> p g d", p=128).bitcast(F32R)
            for gi in range(n_groups):
                nc.sync.dma_start(vn[:, gi, :], vn_src[:, gi, :])
            ob = cpool.tile([128, 4], F32R, tag="ones")
            nc.sync.dma_start(ob[:], ones_dram[:].bitcast(F32R))

            def stage1(g):
                # mm1: scores[(b,v)=128, a=512], accumulate over 4 d-chunks
                scores = ps_s.tile([128, A], F32, tag="scores")
                for dc in range(4):
                    nc.tensor.matmul(
                        scores[:],
                        lhsT=vt[:, dc, 128 * g : 128 * (g + 1)],
                        rhs=tt[:, dc, :],
                        start=(dc == 0),
                        stop=(dc == 3),
                    )
                # e = exp(scores / TEMP)
                expt = epool.tile([128, A], F32R, tag="expt")
                nc.scalar.activation(
                    expt[:],
                    scores[:],
                    mybir.ActivationFunctionType.Exp,
                    bias=0.0,
                    scale=1.0 / TEMP,
                )
                return expt

            def stage2(g, expt):
                # Z[a, b'] = sum_v e[(b',v), a] via block-diagonal ones
                z = ps_z.tile([128, 16], F32, tag="z")
                for ac in range(4):
                    nc.tensor.matmul(
                        z[:, 4 * ac : 4 * (ac + 1)],
                        lhsT=expt[:, 128 * ac : 128 * (ac + 1)],
                        rhs=ob[:],
                        start=True,
                        stop=True,
                    )
                r = rpool.tile([128, 16], F32, tag="r")
                nc.vector.reciprocal(r[:], z[:])

                # mm2 + fused normalize/evict + 1 MB stores
                for ac in range(4):
                    ot = opool.tile([128, 4, D], F32, tag="ot")
                    for bp in range(4):
                        pooled = ps_p.tile([128, D], F32, tag="pooled")
                        nc.tensor.matmul(
                            pooled[:],
                            lhsT=expt[
                                32 * bp : 32 * (bp + 1), 128 * ac : 128 * (ac + 1)
                            ],
                            rhs=vn[32 * bp : 32 * (bp + 1), g, :],
                            start=True,
                            stop=True,
                            tile_position=(32 * bp, 0),
                        )
                        i = 4 * ac + bp
                        if i % 2 == 1:
                            nc.scalar.mul(
                                ot[:, bp, :], pooled[:], mul=r[:, i : i + 1]
                            )
                        else:
                            nc.vector.tensor_scalar_mul(
                                ot[:, bp, :], pooled[:], r[:, i : i + 1]
                            )
                    nc.sync.dma_start(
                        out[128 * ac : 128 * (ac + 1), 4 * g : 4 * (g + 1), :],
                        ot[:],
                    )

            prev = None
            for g in range(n_groups + 1):
                cur = stage1(g) if g < n_groups else None
                if prev is not None:
                    stage2(g - 1, prev)
                prev = cur
    nc.compile()
    return nc


_NC_CACHE: dict[int, bass.Bass] = {}


def _get_nc(n_groups: int = NG) -> bass.Bass:
    if n_groups not in _NC_CACHE:
        _NC_CACHE[n_groups] = build_nc(n_groups)
    return _NC_CACHE[n_groups]


def make_in_maps(text_features: np.ndarray, video_features: np.ndarray):
    text_t = np.ascontiguousarray(np.asarray(text_features, dtype=np.float32).T)
    video_features = np.asarray(video_features, dtype=np.float32)
    in_maps = []
    for c in range(N_CORES):
        vsh = video_features[B_SH * c : B_SH * (c + 1)].reshape(B_SH * V, D)
        in_maps.append(
            {
                "text_t": text_t,
                "video_nat": np.ascontiguousarray(vsh),
                "video_t": np.ascontiguousarray(vsh.T),
            }
        )
    return in_maps


def kernel(text_features: np.ndarray, video_features: np.ndarray) -> np.ndarray:
    nc = _get_nc()
    in_maps = make_in_maps(text_features, video_features)
    res = run_bass_kernel_spmd(nc, in_maps, list(range(N_CORES)))
    return np.concatenate([res.results[c]["out"] for c in range(N_CORES)], axis=1)


# revision 12
# speedup vs baseline: 1.2874x; 1.0126x over previous
"""Trainium2 Bass kernel for AbsolutePool (text-video attention pooling).

reference:
    scores[a,b,v] = sum_d text[a,d] * video[b,v,d]
    w = softmax(scores / 5.0, axis=v)
    out[a,b,d]   = sum_v w[a,b,v] * video[b,v,d]

Shapes: text [A=512, D=512] f32, video [B=512, V=32, D=512] f32,
out [512, 512, 512] f32 (512 MB -> memory-bound on the output write).

Sharding: B is split across the 8 cores (64 b's each).  Each core reads its
4 MB video shard (twice: natural + host-transposed layouts), the replicated
1 MB transposed text, and writes a 64 MB output shard out[:, b_lo:b_hi, :].
The full output is re-assembled on host with one concatenate along axis 1.

Per-core plan (all matmuls in float32r = full-rate FP22 PE mode):
  groups of 4 b's (4*32 v = 128 partitions):
    mm1   scores[(b,v)=128, a=512]  = video_t.T @ text_t   (4 accum steps over d)
    ACT   e = exp(scores / 5)                       PSUM -> SBUF
    mmZ   Z[a=128chunk, 4b] = e.T @ blockdiag_ones  (sum over v per b)
    DVE   R = 1/Z
    mm2   pooled[a=128chunk, d=512] = e_b.T @ video_b   (K = 32 v's)
    DVE/ACT  out_sbuf = pooled * R[a]   (normalize fused into PSUM eviction)
    DMA   out[a_chunk, 4 b's, :]  (1 MB contiguous-per-row stores)
"""

import numpy as np

import concourse.bass as bass
import concourse.bacc as bacc
import concourse.tile as tile
from concourse import mybir
from concourse.bass_utils import run_bass_kernel_spmd

N_CORES = 8
A = 512
D = 512
B = 512
V = 32
B_SH = B // N_CORES            # 64 b's per core
NG = B_SH // 4                 # 16 groups of 4 b's
TEMP = 5.0

F32 = mybir.dt.float32
F32R = mybir.dt.float32r

# which of the 16 (ac, bp) evictions per group go to the scalar engine
# (the rest go to the vector engine; ACT also runs the exp)
ACT_PAIRS = frozenset({1, 4, 6})  # 3 of 8 eviction pairs per group go to ACT


def build_nc(n_groups: int = NG) -> bass.Bass:
    bsh = 4 * n_groups
    nc = bacc.Bacc(None, target_bir_lowering=False, debug=False)

    text_t = nc.dram_tensor("text_t", [D, A], F32, kind="ExternalInput")
    video_nat = nc.dram_tensor("video_nat", [bsh * V, D], F32, kind="ExternalInput")
    video_t = nc.dram_tensor("video_t", [D, bsh * V], F32, kind="ExternalInput")
    out = nc.dram_tensor("out", [A, bsh, D], F32, kind="ExternalOutput")

    ones_np = np.zeros((128, 4), dtype=np.float32)
    for j in range(4):
        ones_np[32 * j : 32 * (j + 1), j] = 1.0
    ones_dram = nc.inline_tensor(ones_np, name="ones_bd")

    with tile.TileContext(nc) as tc:
        with (
            tc.tile_pool(name="const", bufs=1) as cpool,
            tc.tile_pool(name="exp", bufs=3) as epool,
            tc.tile_pool(name="rpool", bufs=3) as rpool,
            tc.tile_pool(name="outp", bufs=8) as opool,
            tc.tile_pool(name="ps_s", bufs=3, space="PSUM") as ps_s,
            tc.tile_pool(name="ps_z", bufs=1, space="PSUM") as ps_z,
            tc.tile_pool(name="ps_p", bufs=4, space="PSUM") as ps_p,
        ):
            # resident inputs ---------------------------------------------
            tt = cpool.tile([128, 4, A], F32R, tag="tt")        # text_t  1 MB
            nc.sync.dma_start(
                tt[:], text_t[:].rearrange("(c p) a -> p c a", p=128).bitcast(F32R)
            )
            vt = cpool.tile([128, 4, bsh * V], F32R, tag="vt")  # video_t 4 MB
            vt_src = video_t[:].rearrange("(c p) n -> p c n", p=128).bitcast(F32R)
            gq = max(1, n_groups // 4)
            for q in range(0, n_groups, gq):
                for dc in range(4):
                    nc.sync.dma_start(
                        vt[:, dc, 128 * q : 128 * (q + gq)],
                        vt_src[:, dc, 128 * q : 128 * (q + gq)],
                    )
            vn = cpool.tile([128, n_groups, D], F32R, tag="vn") # video   4 MB
            vn_src = video_nat[:].rearrange("(g p) d -> p g d", p=128).bitcast(F32R)
            for gi in range(n_groups):
                nc.sync.dma_start(vn[:, gi, :], vn_src[:, gi, :])
            ob = cpool.tile([128, 4], F32R, tag="ones")
            nc.sync.dma_start(ob[:], ones_dram[:].bitcast(F32R))

            last_mm2 = {}

            def stage1(g):
                # mm1: scores[(b,v)=128, a=512], accumulate over 4 d-chunks
                scores = ps_s.tile([128, A], F32, tag="scores")
                for dc in range(4):
                    mm = nc.tensor.matmul(
                        scores[:],
                        lhsT=vt[:, dc, 128 * g : 128 * (g + 1)],
                        rhs=tt[:, dc, :],
                        start=(dc == 0),
                        stop=(dc == 3),
                    )
                    if dc == 0 and g - 2 in last_mm2:
                        # ordering-only dep: keep the PE stream from racing
                        # ahead on mm1's before older groups' mm2/stores run
                        tile.add_dep_helper(
                            mm.ins,
                            last_mm2[g - 2].ins,
                            sync=False,
                            reason="pace mm1 behind stage2 of g-2",
                        )
                # e = exp(scores / TEMP)
                expt = epool.tile([128, A], F32R, tag="expt")
                nc.scalar.activation(
                    expt[:],
                    scores[:],
                    mybir.ActivationFunctionType.Exp,
                    bias=0.0,
                    scale=1.0 / TEMP,
                )
                return expt

            def stage2(g, expt):
                # Z[a, b'] = sum_v e[(b',v), a] via block-diagonal ones
                z = ps_z.tile([128, 16], F32, tag="z")
                for ac in range(4):
                    nc.tensor.matmul(
                        z[:, 4 * ac : 4 * (ac + 1)],
                        lhsT=expt[:, 128 * ac : 128 * (ac + 1)],
                        rhs=ob[:],
                        start=True,
                        stop=True,
                    )
                r = rpool.tile([128, 16], F32, tag="r")
                nc.vector.reciprocal(r[:], z[:])

                # mm2 + fused normalize/evict + 1 MB stores
                for ac in range(4):
                    ot = opool.tile([128, 4, D], F32, tag="ot")
                    for bp in range(4):
                        pooled = ps_p.tile([128, D], F32, tag="pooled")
                        last_mm2[g] = nc.tensor.matmul(
                            pooled[:],
                            lhsT=expt[
                                32 * bp : 32 * (bp + 1), 128 * ac : 128 * (ac + 1)
                            ],
                            rhs=vn[32 * bp : 32 * (bp + 1), g, :],
                            start=True,
                            stop=True,
                            tile_position=(32 * bp, 0),
                        )
                        i = 4 * ac + bp
                        if i % 2 == 1:
                            nc.scalar.mul(
                                ot[:, bp, :], pooled[:], mul=r[:, i : i + 1]
                            )
                        else:
                            nc.vector.tensor_scalar_mul(
                                ot[:, bp, :], pooled[:], r[:, i : i + 1]
                            )
                    nc.sync.dma_start(
                        out[128 * ac : 128 * (ac + 1), 4 * g : 4 * (g + 1), :],
                        ot[:],
                    )

            prev = None
            for g in range(n_groups + 1):
                cur = stage1(g) if g < n_groups else None
                if prev is not None:
                    stage2(g - 1, prev)
                prev = cur
    nc.compile()
    return nc


_NC_CACHE: dict[int, bass.Bass] = {}


def _get_nc(n_groups: int = NG) -> bass.Bass:
    if n_groups not in _NC_CACHE:
        _NC_CACHE[n_groups] = build_nc(n_groups)
    return _NC_CACHE[n_groups]


def make_in_maps(text_features: np.ndarray, video_features: np.ndarray):
    text_t = np.ascontiguousarray(np.asarray(text_features, dtype=np.float32).T)
    video_features = np.asarray(video_features, dtype=np.float32)
    in_maps = []
    for c in range(N_CORES):
        vsh = video_features[B_SH * c : B_SH * (c + 1)].reshape(B_SH * V, D)
        in_maps.append(
            {
                "text_t": text_t,
                "video_nat": np.ascontiguousarray(vsh),
                "video_t": np.ascontiguousarray(vsh.T),
            }
        )
    return in_maps


def kernel(text_features: np.ndarray, video_features: np.ndarray) -> np.ndarray:
    nc = _get_nc()
    in_maps = make_in_maps(text_features, video_features)
    res = run_bass_kernel_spmd(nc, in_maps, list(range(N_CORES)))
    return np.concatenate([res.results[c]["out"] for c in range(N_CORES)], axis=1)
